# revision 12
# baseline (speedup 1.0000x reference)
"""Additive attention on 8 TRN2 cores — fp8 DoubleRow (x-stationary) +
top-|wv| bf16 h-tile; fp8-side score reduction offloaded to DVE.

z-stream structure (per 512-token chunk, h permuted by |wv_h| descending
on the host so tile 0 holds the largest-|wv| 128 h's):
  - h-tiles HB..7: x8 @ W8 in fp8e4 perf_mode=DoubleRow (2 k-blocks per
    512-cycle matmul -> 2x MACs)  [8 DR matmuls each]
  - h-tiles 0..HB-1: exact bf16 x @ W                [16 matmuls each]
Score error is sum_h wv_h^2-weighted, and the top 128 |wv_h| carry 49%
of sum wv^2 (top 256: 71%), so spending bf16 exactness there buys far
more accuracy per cycle than spreading it across k-blocks.

The remaining fp8 quantization error is mostly cancelled by a
host-computed per-token bias riding the exp() activation bias port:
    score_est[t] = sum_h wv_h tanh(zhat_h[t]) + c*(u.x[t] - u_eff.x_eff[t])
with u = wv@W exact, u_eff.x_eff the same rank-1 functional on the
quantized operands the device uses, c ~ E[tanh'(z)] = 0.48.

Numpy sim rel err (same inputs as the harness): HB=1 -> 1.34e-2,
HB=2 -> 1.04e-2 (tolerance 2e-2; all-bf16 baseline 3.5e-3).
Measured on TRN2 (8 cores SPMD): 292.8us HW exec, rel err 1.3387e-2
(sim and HW agree to ~0.1% relative on the error statistic), vs the
all-bf16 baseline's 488us / 3.5e-3.  PE busy 263us (p50 pitch 189ns on
the 448-col DR matmuls, gaps ~8us); the fp8-side score reduction runs
on DVE (scalar_tensor_tensor accum; tensor_tensor_reduce faults on
this hw) so the PE only does 4 tr matmuls per chunk for the bf16 tile;
preamble ~15us is runtime/DMA-ring init.

fp8 scaling: x*16, W*32 pre-cast (normal range, max ~90/5 vs limit
240); tanh scale=1/512 undoes it; bf16 operands carry the same exact
power-of-2 scales.
"""

import numpy as np
import ml_dtypes

P = 128
CORES = 8
CHUNK_W = 512
HB = 1            # top-|wv| h-tiles computed in bf16
C_FIX = 0.48
SX, SW = 16.0, 32.0

FULL_S, FULL_B, FULL_QK, FULL_H, FULL_DV = 1024, 64, 1024, 1024, 1024


def build_nc(S=FULL_S, BL=FULL_B // CORES, QK2=2 * FULL_QK, H=FULL_H,
             DV=FULL_DV, CW=CHUNK_W, hb=HB, XT_BUFS=4, Z_BUFS=3):
    import concourse.mybir as mybir
    import concourse.tile as tile
    from concourse import bacc

    dt = mybir.dt
    f32, bf16, f8 = dt.float32, dt.bfloat16, dt.float8e4
    AF = mybir.ActivationFunctionType
    PM = mybir.MatmulPerfMode

    SB = S * BL
    KO = QK2 // P        # 16 k-blocks
    KP = KO // 2         # 8 DoubleRow pairs
    HT = H // P          # 8 h-tiles; tiles 0..hb-1 are bf16
    HBC = hb * P
    H8 = H - HBC         # fp8 h columns
    CH = SB // CW
    SBLK = S // P
    OCW = min(CW, DV)
    DT = DV // OCW
    TPC = CW // P
    SH = S // CW
    NQ8 = KO // 4        # xt8 tiles per chunk (4 blocks = 2 pairs each)
    NQB = 2              # xbf tiles per chunk (8 blocks each)
    NSW = 2              # fp8 h sweeps per token block
    SWW = H8 // NSW      # sweep width (448)
    AL = mybir.AluOpType
    assert SB % CW == 0 and S % CW == 0 and hb == 1

    nc = bacc.Bacc("TRN2", debug=False, target_bir_lowering=False)

    xt8 = nc.dram_tensor("xt8", [CH, P, KO, CW], f8, kind="ExternalInput").ap()
    xbf = nc.dram_tensor("xbf", [CH, P, KO, CW], bf16, kind="ExternalInput").ap()
    wct8 = nc.dram_tensor("wct8", [P, KO, H8], f8, kind="ExternalInput").ap()
    wcb = nc.dram_tensor("wcb", [P, KO, HBC], bf16, kind="ExternalInput").ap()
    # col 0: tile-0 wv column; col 1: ones (softmax denominator)
    wv0 = nc.dram_tensor("wv0", [P, 2], bf16, kind="ExternalInput").ap()
    # fp8-side wv row, replicated on every partition (DVE elementwise mult)
    wv8r = nc.dram_tensor("wv8r", [P, H8], bf16, kind="ExternalInput").ap()
    vals = nc.dram_tensor("vals", [P, SBLK, DV], bf16, kind="ExternalInput").ap()
    bias = nc.dram_tensor("bias", [P, SBLK, BL], f32, kind="ExternalInput").ap()
    out = nc.dram_tensor("out", [BL, DV], f32, kind="ExternalOutput").ap()

    with tile.TileContext(nc) as tc:
        with tc.tile_pool(name="const", bufs=1) as const_pool, \
             tc.tile_pool(name="xtp", bufs=XT_BUFS) as xt_pool, \
             tc.tile_pool(name="f8p", bufs=3) as f8_pool, \
             tc.tile_pool(name="f0p", bufs=3) as f0_pool, \
             tc.tile_pool(name="jnk", bufs=1) as jnk_pool, \
             tc.tile_pool(name="s8p", bufs=10) as s8_pool, \
             tc.tile_pool(name="misc", bufs=2) as misc_pool, \
             tc.tile_pool(name="zp8", bufs=2, space="PSUM") as zp8, \
             tc.tile_pool(name="z0p", bufs=1, space="PSUM") as z0p, \
             tc.tile_pool(name="tps", bufs=2, space="PSUM") as tpsum, \
             tc.tile_pool(name="dps", bufs=1, space="PSUM") as dpsum, \
             tc.tile_pool(name="ops", bufs=2, space="PSUM") as opsum:

            wct8_ps = [const_pool.tile([P, 2, H8], f8, name=f"wct8_p{kp}")
                       for kp in range(KP)]
            wcb_ks = [const_pool.tile([P, HBC], bf16, name=f"wcb_k{ko}")
                      for ko in range(KO)]
            wv0_sb = const_pool.tile([P, 2], bf16)
            wv8r_sb = const_pool.tile([P, H8], bf16)
            vals_sb = const_pool.tile([P, SBLK, DV], bf16)
            bias_sb = const_pool.tile([P, SBLK, BL], f32)
            a_sb = const_pool.tile([P, SBLK, BL], bf16)
            ones_col = wv0_sb[:, 1:2]

            def load_chunk(c, t8_only=False):
                b, shalf = c % BL, c // BL
                cc = b * SH + shalf
                t8s = []
                for j in range(NQ8):
                    t = xt_pool.tile([P, 4, CW], f8, tag=f"xt8q{j}",
                                     name=f"xt8_{c}_q{j}")
                    nc.sync.dma_start(t[:], xt8[cc, :, 4 * j:4 * j + 4, :])
                    t8s.append(t)
                if t8_only:
                    return t8s, None
                tbs = []
                for j in range(NQB):
                    KH = KO // NQB
                    t = xt_pool.tile([P, KH, CW], bf16, tag=f"xbfq{j}",
                                     name=f"xbf_{c}_q{j}")
                    nc.sync.dma_start(t[:], xbf[cc, :, KH * j:KH * (j + 1), :])
                    tbs.append(t)
                return t8s, tbs

            # startup order follows first-use time: chunk0 fp8 stream,
            # fp8 weights (first matmuls cycle through every pair within
            # ~2us), chunk1 fp8 stream (needed at +~14us), chunk0 bf16
            # stream (+~12us), bf16 weights, wv row, chunk1 bf16
            def load_xbf(c):
                cc = (c % BL) * SH + c // BL
                tbs = []
                for j in range(NQB):
                    KH = KO // NQB
                    t = xt_pool.tile([P, KH, CW], bf16, tag=f"xbfq{j}",
                                     name=f"xbf_{c}_q{j}")
                    nc.sync.dma_start(t[:], xbf[cc, :, KH * j:KH * (j + 1), :])
                    tbs.append(t)
                return tbs

            t8s0, _ = load_chunk(0, t8_only=True)
            for kp in range(KP):
                nc.sync.dma_start(wct8_ps[kp][:], wct8[:, 2 * kp:2 * kp + 2, :])
            t8s1, _ = load_chunk(1, t8_only=True)
            tbs0 = load_xbf(0)
            for ko in range(KO):
                nc.sync.dma_start(wcb_ks[ko][:], wcb[:, ko, :])
            nc.sync.dma_start(wv0_sb[:], wv0[:])
            nc.sync.dma_start(wv8r_sb[:], wv8r[:])
            nc.sync.dma_start(bias_sb[:], bias[:])
            tiles = {0: (t8s0, tbs0), 1: (t8s1, load_xbf(1))}

            den_psum = dpsum.tile([BL, 1], f32)
            o_psums = [opsum.tile([BL, OCW], f32, tag="o", name=f"o_psum{d}")
                       for d in range(DT)]

            def final_mms(sblk_range):
                for sblk in sblk_range:
                    nc.tensor.matmul(
                        den_psum[:], lhsT=a_sb[:, sblk, :], rhs=ones_col,
                        start=(sblk == 0), stop=(sblk == SBLK - 1),
                        skip_group_check=True)
                for d in range(DT):
                    for sblk in sblk_range:
                        nc.tensor.matmul(
                            o_psums[d][:], lhsT=a_sb[:, sblk, :],
                            rhs=vals_sb[:, sblk, d * OCW:(d + 1) * OCW],
                            start=(sblk == 0), stop=(sblk == SBLK - 1),
                            skip_group_check=True)

            def emit_scores(f0, s8s, shalf, b):
                # tile-0 partial scores on PE (one 128-deep matmul per
                # token block); fp8 partial + host bias arrive via the
                # exp() bias port from the DVE accumulation
                for t in range(TPC):
                    tr_psum = tpsum.tile([P, 1], f32, tag="tr")
                    nc.tensor.matmul(
                        tr_psum[:],
                        lhsT=f0[:, t * P:(t + 1) * P],
                        rhs=wv0_sb[:, 0:1],
                        start=True, stop=True,
                        skip_group_check=True)
                    sblk = shalf * TPC + t
                    nc.scalar.activation(a_sb[:, sblk, b:b + 1], tr_psum[:],
                                         AF.Exp, bias=s8s[t][:])

            prev = None
            for c in range(CH):
                shalf = c // BL
                b = c % BL
                # prefetch two chunks ahead (chunks 0/1 preloaded above)
                if c + 2 < CH:
                    tiles[c + 2] = load_chunk(c + 2)
                if c == 2:
                    nc.sync.dma_start(vals_sb[:], vals[:])
                t8s, tbs = tiles.pop(c)

                # fp8 part, x-stationary: z lands [token, h]; per token
                # block, sweep-outer so the tanh of sweep 0 hides under
                # sweep 1's matmuls
                s8s = []
                for t in range(TPC):
                    f8t = f8_pool.tile([P, H8], bf16, tag="f8",
                                       name=f"f8_{c}_{t}")
                    for sw in range(NSW):
                        zp = zp8.tile([P, SWW], f32, tag="z8",
                                      name=f"z8_{c}_{t}_{sw}")
                        for kp in range(KP):
                            nc.tensor.matmul(
                                zp[:],
                                lhsT=t8s[kp // 2][:, 2 * (kp % 2):2 * (kp % 2) + 2,
                                                  t * P:(t + 1) * P],
                                rhs=wct8_ps[kp][:, :, sw * SWW:(sw + 1) * SWW],
                                start=(kp == 0), stop=(kp == KP - 1),
                                perf_mode=PM.DoubleRow,
                                skip_group_check=True)
                        nc.scalar.activation(f8t[:, sw * SWW:(sw + 1) * SWW],
                                             zp[:], AF.Tanh,
                                             scale=1.0 / (SX * SW))
                    # fp8 partial score + host bias, off the PE entirely
                    # (tensor_tensor_reduce faults on this hw; use the
                    # TensorScalarPtr form + a scalar-engine bias add)
                    s8 = s8_pool.tile([P, 1], f32, tag="s8",
                                      name=f"s8_{c}_{t}")
                    g = jnk_pool.tile([P, H8], bf16, tag="g")
                    sblk = shalf * TPC + t
                    nc.vector.scalar_tensor_tensor(
                        out=g[:], in0=f8t[:], scalar=1.0, in1=wv8r_sb[:],
                        op0=AL.mult, op1=AL.mult, accum_out=s8[:])
                    comb = s8_pool.tile([P, 1], f32, tag="comb",
                                        name=f"comb_{c}_{t}")
                    nc.scalar.add(comb[:], s8[:], bias_sb[:, sblk, b:b + 1])
                    s8s.append(comb)

                # bf16 top tile, weights-stationary (as before)
                KH = KO // NQB
                z0 = z0p.tile([P, CW], f32, tag="z0", name=f"z0_{c}")
                for ko in range(KO):
                    nc.tensor.matmul(
                        z0[:],
                        lhsT=wcb_ks[ko][:, 0:P],
                        rhs=tbs[ko // KH][:, ko % KH, :],
                        start=(ko == 0), stop=(ko == KO - 1),
                        skip_group_check=True)
                f0 = f0_pool.tile([P, CW], bf16, tag="f0")
                nc.scalar.activation(f0[:], z0[:], AF.Tanh,
                                     scale=1.0 / (SX * SW))

                if prev is not None:
                    emit_scores(*prev)
                prev = (f0, s8s, shalf, b)
                # one chunk later than strictly needed so the a_sb slices
                # are certainly written and the PE never stalls here
                if c % BL == 1 and c > 1:
                    final_mms(range((shalf - 1) * TPC, shalf * TPC))

            emit_scores(*prev)
            final_mms(range((SH - 1) * TPC, SH * TPC))

            den_inv = misc_pool.tile([BL, 1], f32, tag="dinv")
            nc.vector.reciprocal(den_inv[:], den_psum[:])
            for d in range(DT):
                o_sb = misc_pool.tile([BL, OCW], f32, tag=f"o{d}")
                nc.scalar.activation(o_sb[:], o_psums[d][:], AF.Copy,
                                     scale=den_inv[:])
                nc.sync.dma_start(out[:, d * OCW:(d + 1) * OCW], o_sb[:])
    return nc


def prep_in_maps(queries, keys, values, W_q, W_k, w_v, n_cores=CORES,
                 hb=HB, c_fix=C_FIX):
    bf = ml_dtypes.bfloat16
    e4 = ml_dtypes.float8_e4m3
    queries = np.asarray(queries, dtype=np.float32)
    keys = np.asarray(keys, dtype=np.float32)
    S, B, QK = queries.shape
    BL = B // n_cores
    H = np.asarray(W_q).shape[0]
    HT = H // P
    KO = 2 * QK // P
    CW = CHUNK_W
    CH = S * BL // CW
    SBLK = S // P
    HBC = hb * P

    wvb_raw = np.asarray(w_v, np.float32).reshape(H)
    wvb0 = wvb_raw.astype(bf).astype(np.float32)
    perm = np.argsort(-np.abs(wvb0), kind="stable")

    Wcat = np.concatenate([np.asarray(W_q, np.float32),
                           np.asarray(W_k, np.float32)], axis=1)[perm]  # [H,2QK]
    wvp = wvb0[perm]

    Wb = (Wcat[:HBC] * SW).astype(bf)                     # [HBC, 2QK]
    W8 = (Wcat[HBC:] * SW).astype(e4)                     # [H-HBC, 2QK]
    wcb_np = np.ascontiguousarray(
        Wb.T.reshape(KO, P, HBC).transpose(1, 0, 2))
    wct8_np = np.ascontiguousarray(
        W8.T.reshape(KO, P, H - HBC).transpose(1, 0, 2))

    wv0_np = np.empty((P, 2), dtype=bf)
    wv0_np[:, 0] = wvp[:HBC].astype(bf)
    wv0_np[:, 1] = np.float32(1.0)
    wv8r_np = np.ascontiguousarray(
        np.broadcast_to(wvp[HBC:].astype(bf), (P, H - HBC)))

    DV = np.asarray(values).shape[2]
    vals_np = np.ascontiguousarray(
        np.asarray(values, np.float32)[:, 0, :].astype(bf)
        .reshape(S // P, P, DV).transpose(1, 0, 2))

    u_exact = wvp @ Wcat                                  # [2QK]
    u_b = (wvp[:HBC] @ Wb.astype(np.float32)) / SW
    u_8 = (wvp[HBC:] @ W8.astype(np.float32)) / SW

    in_maps = []
    for cidx in range(n_cores):
        q = queries[:, cidx * BL:(cidx + 1) * BL, :]
        k = keys[:, cidx * BL:(cidx + 1) * BL, :]
        qT = np.ascontiguousarray(q.transpose(2, 1, 0)).reshape(QK, S * BL)
        kT = np.ascontiguousarray(k.transpose(2, 1, 0)).reshape(QK, S * BL)
        xt2d = np.concatenate([qT, kT], axis=0)           # [2QK, SB] f32
        x8 = (xt2d * SX).astype(e4)
        xb = (xt2d * SX).astype(bf)
        xt8_np = np.ascontiguousarray(
            x8.reshape(KO, P, CH, CW).transpose(2, 1, 0, 3))
        xbf_np = np.ascontiguousarray(
            xb.reshape(KO, P, CH, CW).transpose(2, 1, 0, 3))

        dot_exact = u_exact @ xt2d
        dot_eff = (u_b @ (xb.astype(np.float32) / SX)
                   + u_8 @ (x8.astype(np.float32) / SX))
        r = c_fix * (dot_exact - dot_eff)
        bias_np = np.ascontiguousarray(
            r.reshape(BL, SBLK, P).transpose(2, 1, 0)).astype(np.float32)

        in_maps.append({"xt8": xt8_np, "xbf": xbf_np, "wct8": wct8_np,
                        "wcb": wcb_np, "wv0": wv0_np, "wv8r": wv8r_np,
                        "vals": vals_np, "bias": bias_np})
    return in_maps


_NC_CACHE = {}


def _get_nc():
    if "nc" not in _NC_CACHE:
        nc = build_nc()
        nc.finalize()
        _NC_CACHE["nc"] = nc
    return _NC_CACHE["nc"]


def kernel_with_results(trace=False, **inputs):
    from concourse.bass_utils import run_bass_kernel_spmd
    nc = _get_nc()
    in_maps = prep_in_maps(**inputs)
    res = run_bass_kernel_spmd(nc, in_maps, core_ids=list(range(CORES)),
                               trace=trace)
    out = np.concatenate([np.asarray(res.results[i]["out"], np.float32)
                          for i in range(CORES)], axis=0)
    return out, res


def kernel(**inputs):
    out, _ = kernel_with_results(trace=False, **inputs)
    return out


# revision 14
# speedup vs baseline: 1.1909x; 1.1909x over previous
"""Additive attention on 8 TRN2 cores — fp8 DoubleRow (x-stationary) +
top-|wv| bf16 h-tile; fp8-side score reduction offloaded to DVE.

z-stream structure (per 512-token chunk, h permuted by |wv_h| descending
on the host so tile 0 holds the largest-|wv| 128 h's):
  - h-tiles HB..7: x8 @ W8 in fp8e4 perf_mode=DoubleRow (2 k-blocks per
    512-cycle matmul -> 2x MACs)  [8 DR matmuls each]
  - h-tiles 0..HB-1: exact bf16 x @ W                [16 matmuls each]
Score error is sum_h wv_h^2-weighted, and the top 128 |wv_h| carry 49%
of sum wv^2 (top 256: 71%), so spending bf16 exactness there buys far
more accuracy per cycle than spreading it across k-blocks.

The remaining fp8 quantization error is mostly cancelled by a
host-computed per-token bias riding the exp() activation bias port:
    score_est[t] = sum_h wv_h tanh(zhat_h[t]) + c*(u.x[t] - u_eff.x_eff[t])
with u = wv@W exact, u_eff.x_eff the same rank-1 functional on the
quantized operands the device uses, c ~ E[tanh'(z)] = 0.48.

Numpy sim rel err (same inputs as the harness): HB=1 -> 1.34e-2,
HB=2 -> 1.04e-2 (tolerance 2e-2; all-bf16 baseline 3.5e-3).
Measured on TRN2 (8 cores SPMD): 292.8us HW exec at the 2.37GHz clock
state, rel err 1.3387e-2 (sim and HW agree to ~0.1% relative on the
error statistic), vs the all-bf16 baseline's 488us / 3.5e-3.  PE busy
263us: 1024 DoubleRow matmuls (448 cols, p50 189ns) + 256 bf16 top-tile
matmuls + 4 tr matmuls/chunk; fp8-side score reduction rides DVE
(scalar_tensor_tensor accum — NB tensor_tensor_reduce faults this hw).
Caveat: the chip clock is bimodal under sustained benching (~2.0GHz hot
= +18-20% wall on the identical NEFF); compare A/B timings only within
the same thermal state (check the DR matmul pitch: 189ns vs ~227ns).

fp8 scaling: x*16, W*32 pre-cast (normal range, max ~90/5 vs limit
240); tanh scale=1/512 undoes it; bf16 operands carry the same exact
power-of-2 scales.
"""

import numpy as np
import ml_dtypes

P = 128
CORES = 8
CHUNK_W = 512
HB = 1            # top-|wv| h-tiles computed in bf16
C_FIX = 0.48
SX, SW = 16.0, 32.0

FULL_S, FULL_B, FULL_QK, FULL_H, FULL_DV = 1024, 64, 1024, 1024, 1024


def build_nc(S=FULL_S, BL=FULL_B // CORES, QK2=2 * FULL_QK, H=FULL_H,
             DV=FULL_DV, CW=CHUNK_W, hb=HB, XT_BUFS=4, Z_BUFS=3):
    import concourse.mybir as mybir
    import concourse.tile as tile
    from concourse import bacc

    dt = mybir.dt
    f32, bf16, f8 = dt.float32, dt.bfloat16, dt.float8e4
    AF = mybir.ActivationFunctionType
    PM = mybir.MatmulPerfMode

    SB = S * BL
    KO = QK2 // P        # 16 k-blocks
    KP = KO // 2         # 8 DoubleRow pairs
    HT = H // P          # 8 h-tiles; tiles 0..hb-1 are bf16
    HBC = hb * P
    H8 = H - HBC         # fp8 h columns
    CH = SB // CW
    SBLK = S // P
    OCW = min(CW, DV)
    DT = DV // OCW
    TPC = CW // P
    SH = S // CW
    NQ8 = KO // 4        # xt8 tiles per chunk (4 blocks = 2 pairs each)
    NQB = 2              # xbf tiles per chunk (8 blocks each)
    NSW = 2              # fp8 h sweeps per token block
    SWW = H8 // NSW      # sweep width (448)
    AL = mybir.AluOpType
    assert SB % CW == 0 and S % CW == 0 and hb == 1

    nc = bacc.Bacc("TRN2", debug=False, target_bir_lowering=False)

    xt8 = nc.dram_tensor("xt8", [CH, P, KO, CW], f8, kind="ExternalInput").ap()
    xbf = nc.dram_tensor("xbf", [CH, P, KO, CW], bf16, kind="ExternalInput").ap()
    wct8 = nc.dram_tensor("wct8", [P, KO, H8], f8, kind="ExternalInput").ap()
    wcb = nc.dram_tensor("wcb", [P, KO, HBC], bf16, kind="ExternalInput").ap()
    # col 0: tile-0 wv column; col 1: ones (softmax denominator)
    wv0 = nc.dram_tensor("wv0", [P, 2], bf16, kind="ExternalInput").ap()
    # fp8-side wv row, replicated on every partition (DVE elementwise mult)
    wv8r = nc.dram_tensor("wv8r", [P, H8], bf16, kind="ExternalInput").ap()
    vals = nc.dram_tensor("vals", [P, SBLK, DV], bf16, kind="ExternalInput").ap()
    bias = nc.dram_tensor("bias", [P, SBLK, BL], f32, kind="ExternalInput").ap()
    out = nc.dram_tensor("out", [BL, DV], f32, kind="ExternalOutput").ap()

    with tile.TileContext(nc) as tc:
        with tc.tile_pool(name="const", bufs=1) as const_pool, \
             tc.tile_pool(name="xtp", bufs=XT_BUFS) as xt_pool, \
             tc.tile_pool(name="f8p", bufs=3) as f8_pool, \
             tc.tile_pool(name="f0p", bufs=3) as f0_pool, \
             tc.tile_pool(name="jnk", bufs=1) as jnk_pool, \
             tc.tile_pool(name="s8p", bufs=10) as s8_pool, \
             tc.tile_pool(name="misc", bufs=2) as misc_pool, \
             tc.tile_pool(name="zp8", bufs=2, space="PSUM") as zp8, \
             tc.tile_pool(name="z0p", bufs=1, space="PSUM") as z0p, \
             tc.tile_pool(name="tps", bufs=2, space="PSUM") as tpsum, \
             tc.tile_pool(name="dps", bufs=1, space="PSUM") as dpsum, \
             tc.tile_pool(name="ops", bufs=2, space="PSUM") as opsum:

            wct8_ps = [const_pool.tile([P, 2, H8], f8, name=f"wct8_p{kp}")
                       for kp in range(KP)]
            wcb_ks = [const_pool.tile([P, HBC], bf16, name=f"wcb_k{ko}")
                      for ko in range(KO)]
            wv0_sb = const_pool.tile([P, 2], bf16)
            wv8r_sb = const_pool.tile([P, H8], bf16)
            vals_sb = const_pool.tile([P, SBLK, DV], bf16)
            bias_sb = const_pool.tile([P, SBLK, BL], f32)
            a_sb = const_pool.tile([P, SBLK, BL], bf16)
            ones_col = wv0_sb[:, 1:2]

            def load_chunk(c, t8_only=False):
                b, shalf = c % BL, c // BL
                cc = b * SH + shalf
                t8s = []
                for j in range(NQ8):
                    t = xt_pool.tile([P, 4, CW], f8, tag=f"xt8q{j}",
                                     name=f"xt8_{c}_q{j}")
                    nc.sync.dma_start(t[:], xt8[cc, :, 4 * j:4 * j + 4, :])
                    t8s.append(t)
                if t8_only:
                    return t8s, None
                tbs = []
                for j in range(NQB):
                    KH = KO // NQB
                    t = xt_pool.tile([P, KH, CW], bf16, tag=f"xbfq{j}",
                                     name=f"xbf_{c}_q{j}")
                    nc.sync.dma_start(t[:], xbf[cc, :, KH * j:KH * (j + 1), :])
                    tbs.append(t)
                return t8s, tbs

            # startup: chunk0 fp8 stream, then all fp8 weights (first
            # matmuls cycle through every pair within ~2us), then the
            # bf16 side, then chunk0 bf16 stream
            t8s0, _ = load_chunk(0, t8_only=True)
            for kp in range(KP):
                nc.sync.dma_start(wct8_ps[kp][:], wct8[:, 2 * kp:2 * kp + 2, :])
            for ko in range(KO):
                nc.sync.dma_start(wcb_ks[ko][:], wcb[:, ko, :])
            tbs0 = []
            for j in range(NQB):
                KH = KO // NQB
                t = xt_pool.tile([P, KH, CW], bf16, tag=f"xbfq{j}",
                                 name=f"xbf_0_q{j}")
                nc.sync.dma_start(t[:], xbf[0, :, KH * j:KH * (j + 1), :])
                tbs0.append(t)
            tiles = {0: (t8s0, tbs0)}
            nc.sync.dma_start(wv0_sb[:], wv0[:])
            nc.sync.dma_start(wv8r_sb[:], wv8r[:])
            nc.sync.dma_start(bias_sb[:], bias[:])

            den_psum = dpsum.tile([BL, 1], f32)
            o_psums = [opsum.tile([BL, OCW], f32, tag="o", name=f"o_psum{d}")
                       for d in range(DT)]

            def final_mms(sblk_range):
                for sblk in sblk_range:
                    nc.tensor.matmul(
                        den_psum[:], lhsT=a_sb[:, sblk, :], rhs=ones_col,
                        start=(sblk == 0), stop=(sblk == SBLK - 1),
                        skip_group_check=True)
                for d in range(DT):
                    for sblk in sblk_range:
                        nc.tensor.matmul(
                            o_psums[d][:], lhsT=a_sb[:, sblk, :],
                            rhs=vals_sb[:, sblk, d * OCW:(d + 1) * OCW],
                            start=(sblk == 0), stop=(sblk == SBLK - 1),
                            skip_group_check=True)

            def emit_scores(f0, s8s, shalf, b):
                # tile-0 partial scores on PE (one 128-deep matmul per
                # token block); fp8 partial + host bias arrive via the
                # exp() bias port from the DVE accumulation
                for t in range(TPC):
                    tr_psum = tpsum.tile([P, 1], f32, tag="tr")
                    nc.tensor.matmul(
                        tr_psum[:],
                        lhsT=f0[:, t * P:(t + 1) * P],
                        rhs=wv0_sb[:, 0:1],
                        start=True, stop=True,
                        skip_group_check=True)
                    sblk = shalf * TPC + t
                    nc.scalar.activation(a_sb[:, sblk, b:b + 1], tr_psum[:],
                                         AF.Exp, bias=s8s[t][:])

            prev = None
            for c in range(CH):
                shalf = c // BL
                b = c % BL
                if c + 1 < CH:
                    tiles[c + 1] = load_chunk(c + 1)
                if c == 2:
                    nc.sync.dma_start(vals_sb[:], vals[:])
                t8s, tbs = tiles.pop(c)

                # fp8 part, x-stationary: z lands [token, h]; per token
                # block, sweep-outer so the tanh of sweep 0 hides under
                # sweep 1's matmuls
                s8s = []
                for t in range(TPC):
                    f8t = f8_pool.tile([P, H8], bf16, tag="f8",
                                       name=f"f8_{c}_{t}")
                    for sw in range(NSW):
                        zp = zp8.tile([P, SWW], f32, tag="z8",
                                      name=f"z8_{c}_{t}_{sw}")
                        for kp in range(KP):
                            nc.tensor.matmul(
                                zp[:],
                                lhsT=t8s[kp // 2][:, 2 * (kp % 2):2 * (kp % 2) + 2,
                                                  t * P:(t + 1) * P],
                                rhs=wct8_ps[kp][:, :, sw * SWW:(sw + 1) * SWW],
                                start=(kp == 0), stop=(kp == KP - 1),
                                perf_mode=PM.DoubleRow,
                                skip_group_check=True)
                        nc.scalar.activation(f8t[:, sw * SWW:(sw + 1) * SWW],
                                             zp[:], AF.Tanh,
                                             scale=1.0 / (SX * SW))
                    # fp8 partial score + host bias, off the PE entirely
                    # (tensor_tensor_reduce faults on this hw; use the
                    # TensorScalarPtr form + a scalar-engine bias add)
                    s8 = s8_pool.tile([P, 1], f32, tag="s8",
                                      name=f"s8_{c}_{t}")
                    g = jnk_pool.tile([P, H8], bf16, tag="g")
                    sblk = shalf * TPC + t
                    nc.vector.scalar_tensor_tensor(
                        out=g[:], in0=f8t[:], scalar=1.0, in1=wv8r_sb[:],
                        op0=AL.mult, op1=AL.mult, accum_out=s8[:])
                    comb = s8_pool.tile([P, 1], f32, tag="comb",
                                        name=f"comb_{c}_{t}")
                    nc.scalar.add(comb[:], s8[:], bias_sb[:, sblk, b:b + 1])
                    s8s.append(comb)

                # bf16 top tile, weights-stationary (as before)
                KH = KO // NQB
                z0 = z0p.tile([P, CW], f32, tag="z0", name=f"z0_{c}")
                for ko in range(KO):
                    nc.tensor.matmul(
                        z0[:],
                        lhsT=wcb_ks[ko][:, 0:P],
                        rhs=tbs[ko // KH][:, ko % KH, :],
                        start=(ko == 0), stop=(ko == KO - 1),
                        skip_group_check=True)
                f0 = f0_pool.tile([P, CW], bf16, tag="f0")
                nc.scalar.activation(f0[:], z0[:], AF.Tanh,
                                     scale=1.0 / (SX * SW))

                if prev is not None:
                    emit_scores(*prev)
                prev = (f0, s8s, shalf, b)
                # one chunk later than strictly needed so the a_sb slices
                # are certainly written and the PE never stalls here
                if c % BL == 1 and c > 1:
                    final_mms(range((shalf - 1) * TPC, shalf * TPC))

            emit_scores(*prev)
            final_mms(range((SH - 1) * TPC, SH * TPC))

            den_inv = misc_pool.tile([BL, 1], f32, tag="dinv")
            nc.vector.reciprocal(den_inv[:], den_psum[:])
            for d in range(DT):
                o_sb = misc_pool.tile([BL, OCW], f32, tag=f"o{d}")
                nc.scalar.activation(o_sb[:], o_psums[d][:], AF.Copy,
                                     scale=den_inv[:])
                nc.sync.dma_start(out[:, d * OCW:(d + 1) * OCW], o_sb[:])
    return nc


def prep_in_maps(queries, keys, values, W_q, W_k, w_v, n_cores=CORES,
                 hb=HB, c_fix=C_FIX):
    bf = ml_dtypes.bfloat16
    e4 = ml_dtypes.float8_e4m3
    queries = np.asarray(queries, dtype=np.float32)
    keys = np.asarray(keys, dtype=np.float32)
    S, B, QK = queries.shape
    BL = B // n_cores
    H = np.asarray(W_q).shape[0]
    HT = H // P
    KO = 2 * QK // P
    CW = CHUNK_W
    CH = S * BL // CW
    SBLK = S // P
    HBC = hb * P

    wvb_raw = np.asarray(w_v, np.float32).reshape(H)
    wvb0 = wvb_raw.astype(bf).astype(np.float32)
    perm = np.argsort(-np.abs(wvb0), kind="stable")

    Wcat = np.concatenate([np.asarray(W_q, np.float32),
                           np.asarray(W_k, np.float32)], axis=1)[perm]  # [H,2QK]
    wvp = wvb0[perm]

    Wb = (Wcat[:HBC] * SW).astype(bf)                     # [HBC, 2QK]
    W8 = (Wcat[HBC:] * SW).astype(e4)                     # [H-HBC, 2QK]
    wcb_np = np.ascontiguousarray(
        Wb.T.reshape(KO, P, HBC).transpose(1, 0, 2))
    wct8_np = np.ascontiguousarray(
        W8.T.reshape(KO, P, H - HBC).transpose(1, 0, 2))

    wv0_np = np.empty((P, 2), dtype=bf)
    wv0_np[:, 0] = wvp[:HBC].astype(bf)
    wv0_np[:, 1] = np.float32(1.0)
    wv8r_np = np.ascontiguousarray(
        np.broadcast_to(wvp[HBC:].astype(bf), (P, H - HBC)))

    DV = np.asarray(values).shape[2]
    vals_np = np.ascontiguousarray(
        np.asarray(values, np.float32)[:, 0, :].astype(bf)
        .reshape(S // P, P, DV).transpose(1, 0, 2))

    u_exact = wvp @ Wcat                                  # [2QK]
    u_b = (wvp[:HBC] @ Wb.astype(np.float32)) / SW
    u_8 = (wvp[HBC:] @ W8.astype(np.float32)) / SW

    in_maps = []
    for cidx in range(n_cores):
        q = queries[:, cidx * BL:(cidx + 1) * BL, :]
        k = keys[:, cidx * BL:(cidx + 1) * BL, :]
        qT = np.ascontiguousarray(q.transpose(2, 1, 0)).reshape(QK, S * BL)
        kT = np.ascontiguousarray(k.transpose(2, 1, 0)).reshape(QK, S * BL)
        xt2d = np.concatenate([qT, kT], axis=0)           # [2QK, SB] f32
        x8 = (xt2d * SX).astype(e4)
        xb = (xt2d * SX).astype(bf)
        xt8_np = np.ascontiguousarray(
            x8.reshape(KO, P, CH, CW).transpose(2, 1, 0, 3))
        xbf_np = np.ascontiguousarray(
            xb.reshape(KO, P, CH, CW).transpose(2, 1, 0, 3))

        dot_exact = u_exact @ xt2d
        dot_eff = (u_b @ (xb.astype(np.float32) / SX)
                   + u_8 @ (x8.astype(np.float32) / SX))
        r = c_fix * (dot_exact - dot_eff)
        bias_np = np.ascontiguousarray(
            r.reshape(BL, SBLK, P).transpose(2, 1, 0)).astype(np.float32)

        in_maps.append({"xt8": xt8_np, "xbf": xbf_np, "wct8": wct8_np,
                        "wcb": wcb_np, "wv0": wv0_np, "wv8r": wv8r_np,
                        "vals": vals_np, "bias": bias_np})
    return in_maps


_NC_CACHE = {}


def _get_nc():
    if "nc" not in _NC_CACHE:
        nc = build_nc()
        nc.finalize()
        _NC_CACHE["nc"] = nc
    return _NC_CACHE["nc"]


def kernel_with_results(trace=False, **inputs):
    from concourse.bass_utils import run_bass_kernel_spmd
    nc = _get_nc()
    in_maps = prep_in_maps(**inputs)
    res = run_bass_kernel_spmd(nc, in_maps, core_ids=list(range(CORES)),
                               trace=trace)
    out = np.concatenate([np.asarray(res.results[i]["out"], np.float32)
                          for i in range(CORES)], axis=0)
    return out, res


def kernel(**inputs):
    out, _ = kernel_with_results(trace=False, **inputs)
    return out
